# revision 7
# baseline (speedup 1.0000x reference)
"""Multi-head causal attention (B=2, S=2048, D=1024, H=16) on 8 TRN2 cores.

Sharding: data-parallel over batch (2) x tensor-parallel over heads (4 groups
of 4 heads). Core i handles batch i//4, heads 4*(i%4) .. 4*(i%4)+4. Each core
computes the partial output sum_{its heads} softmax(mask(q k^T)/8) v @ Wo_rows
as a full [2048, 1024] array; the host sums the 4 partials per batch and adds
the bias.

Device kernel (identical SPMD program, per-core data):
  inputs : xT [1024, 2048] (x[b] transposed), wq/wk/wv [1024, 256] (head cols),
           wo [256, 1024] (head rows), mask [128, 2048] (causal 0/1 diag band)
  output : out [2048, 1024] partial

All matmuls run as float32r (full-rate fp32 PE mode). Attention works in the
transposed layout S_T[kj, qi] so that:
  - scores matmul: lhsT = K_T block [64, 128], rhs = Q_T chunk [64, 512]
  - softmax exp is fused into the PSUM->SBUF copy on the scalar engine
    (exp(s/8), no max subtraction needed: |s|/8 <= ~3 for this data scale)
  - P @ V needs no transposes: out O_T[d, qi] = sum_kb V[kb]^T P_T[kb]
  - the softmax denominator comes free as an extra ones-column in V
  - normalization is a per-qi-column scale, broadcast across partitions with a
    K=1 outer-product matmul, applied on the PSUM->SBUF copy of O_T
"""

import os
import sys

for _p in ("/opt/trn_rl_repo", "/root/.axon_site/_ro/trn_rl_repo"):
    if os.path.isdir(_p) and _p not in sys.path:
        sys.path.insert(0, _p)

import numpy as np

import concourse.bass as bass
import concourse.tile as tile
from concourse import bacc
from concourse import mybir
from concourse.mybir import dt

# Problem shape (hardcoded per contract)
B, S, D, H = 2, 2048, 1024, 16
HD = D // H          # 64 head dim
NH = 4               # heads per core
P = 128              # partitions
KSUB = D // P        # 8 contraction subtiles
SB = S // P          # 16 seq blocks
QC = 512             # qi chunk width
NCH = S // QC        # 4 qi chunks
VW = 65 * NH         # v_all row width: per head [V(64) | ones(1)]

RD = dt.float32r   # "rounded" fp32: full-rate PE mode; dt.float32 = exact but 4x slower

F32 = dt.float32
Exp = mybir.ActivationFunctionType.Exp


def build_nc():
    nc = bacc.Bacc("TRN2", target_bir_lowering=False, debug=False)

    xT_d = nc.declare_dram_parameter("xT", [D, S], RD, isOutput=False)
    wq_d = nc.declare_dram_parameter("wq", [D, NH * HD], RD, isOutput=False)
    wk_d = nc.declare_dram_parameter("wk", [D, NH * HD], RD, isOutput=False)
    wv_d = nc.declare_dram_parameter("wv", [D, NH * HD], RD, isOutput=False)
    wo_d = nc.declare_dram_parameter("wo", [NH * HD, D], RD, isOutput=False)
    mask_d = nc.declare_dram_parameter("mask", [P, 4 * QC], F32, isOutput=False)
    out_d = nc.declare_dram_parameter("out", [S, D], F32, isOutput=True)

    with tile.TileContext(nc) as tc:
        with nc.allow_low_precision(reason="float32r matmul fast path"):
            _build_body(tc, xT_d, wq_d, wk_d, wv_d, wo_d, mask_d, out_d)
    nc.compile()
    return nc


def _build_body(tc, xT_d, wq_d, wk_d, wv_d, wo_d, mask_d, out_d):
    nc = tc.nc
    from contextlib import ExitStack

    with ExitStack() as ctx:
        persist = ctx.enter_context(tc.tile_pool(name="persist", bufs=1))
        work = ctx.enter_context(tc.tile_pool(name="work", bufs=1))

        # ---- persistent tiles -------------------------------------------
        # Q_T / K_T per head pair g: [128, S]; partitions [0:64] = head 2g,
        # [64:128] = head 2g+1 (falls straight out of the projection matmul).
        qt = [persist.tile([P, S], RD, tag=f"qt{g}", name=f"qt{g}") for g in range(2)]
        kt = [persist.tile([P, S], RD, tag=f"kt{g}", name=f"kt{g}") for g in range(2)]
        # V layout [kj part, sblock, head-interleaved cols]: per head
        # [V_h (64) | ones (1)] so the PV matmul's lhsT [128, 65] yields
        # O_T rows 0:64 = z_h^T and row 64 = softmax denominator.
        vall = persist.tile([P, SB, VW], RD, tag="vall")
        # z^T per head [64, S] (d on partitions 0:64)
        zt = [persist.tile([HD, S], RD, tag=f"zt{h}", name=f"zt{h}") for h in range(NH)]
        mask_sb = persist.tile([P, 4, QC], F32, tag="mask")
        wo_sb = persist.tile([HD, NH, D], RD, tag="wo")
        ones_sb = persist.tile([P, HD], RD, tag="ones")

        # memset cannot emit float32r; stage 1.0 in f32 and round via DVE copy
        onesf = persist.tile([P, 1], F32, tag="onesf")
        nc.vector.memset(onesf[:], 1.0)
        nc.vector.tensor_copy(ones_sb[:], onesf[:].to_broadcast((P, HD)))
        nc.vector.tensor_copy(
            vall[:].rearrange("p s (h c) -> p s h c", c=65)[:, :, :, 64],
            onesf[:].to_broadcast((P, SB, NH)),
        )
        nc.sync.dma_start(mask_sb[:], mask_d[:].rearrange("p (j q) -> p j q", q=QC))
        nc.sync.dma_start(wo_sb[:], wo_d[:].rearrange("(h p) n -> p h n", p=HD))

        # ---- phase 1: projections ---------------------------------------
        with (
            tc.tile_pool(name="pin", bufs=1) as pin,
            tc.tile_pool(name="ppsum", bufs=4, space="PSUM") as ppsum,
        ):
            xt_sb = pin.tile([P, KSUB, S], RD, tag="xt")
            for ko in range(KSUB):
                nc.sync.dma_start(
                    xt_sb[:, ko, :], xT_d[ko * P : (ko + 1) * P, :]
                )
            w_sb = {}
            for name, d_ap in (("wq", wq_d), ("wk", wk_d), ("wv", wv_d)):
                w = pin.tile([P, KSUB, NH * HD], RD, tag=name)
                nc.sync.dma_start(
                    w[:], d_ap[:].rearrange("(ko p) c -> p ko c", p=P)
                )
                w_sb[name] = w

            # Q_T, K_T: out [128 (2 heads), s chunk 512] = W_pair^T @ xT
            for dst, wname in ((qt, "wq"), (kt, "wk")):
                w = w_sb[wname]
                for g in range(2):
                    for sc in range(NCH):
                        ps = ppsum.tile([P, QC], F32, tag="pp")
                        for ko in range(KSUB):
                            nc.tensor.matmul(
                                ps[:],
                                (w[:, ko, g * P : (g + 1) * P]),
                                (xt_sb[:, ko, sc * QC : (sc + 1) * QC]),
                                start=(ko == 0),
                                stop=(ko == KSUB - 1),
                            )
                        nc.vector.tensor_copy(
                            dst[g][:, sc * QC : (sc + 1) * QC], ps[:]
                        )

            # V: out [s block 128, 4*64] = xT_block^T @ Wv, scattered into
            # the 65-stride vall layout.
            wv = w_sb["wv"]
            for sb in range(SB):
                ps = ppsum.tile([P, NH * HD], F32, tag="pv")
                for ko in range(KSUB):
                    nc.tensor.matmul(
                        ps[:],
                        (xt_sb[:, ko, sb * P : (sb + 1) * P]),
                        (wv[:, ko, :]),
                        start=(ko == 0),
                        stop=(ko == KSUB - 1),
                    )
                nc.vector.tensor_copy(
                    vall[:, sb, :].rearrange("p (h c) -> p h c", c=65)[:, :, 0:64],
                    ps[:].rearrange("p (h c) -> p h c", c=HD),
                )

        # ---- phase 2: attention -----------------------------------------
        with (
            tc.tile_pool(name="spool", bufs=2, space="PSUM") as spool,
            tc.tile_pool(name="opool", bufs=3, space="PSUM") as opool,
            tc.tile_pool(name="ptpool", bufs=4) as ptpool,
            tc.tile_pool(name="nrm", bufs=2) as nrm,
        ):
            for g in range(2):
                for c in range(NCH):
                    nkb = 4 * (c + 1)          # causal: kj blocks 0..nkb-1
                    ngrp = nkb // 2
                    for hl in range(2):        # head within pair
                        h = 2 * g + hl
                        q_h = qt[g][hl * HD : (hl + 1) * HD, c * QC : (c + 1) * QC]
                        ot = opool.tile([P, QC], F32, tag="ot")
                        for grp in range(ngrp):
                            sg = spool.tile([P, 2, QC], F32, tag="sg")
                            for j in range(2):
                                kb = 2 * grp + j
                                nc.tensor.matmul(
                                    sg[:, j, :],
                                    (
                                        kt[g][
                                            hl * HD : (hl + 1) * HD,
                                            kb * P : (kb + 1) * P,
                                        ]
                                    ),
                                    (q_h),
                                    start=True,
                                    stop=True,
                                )
                            pt = ptpool.tile([P, 2, QC], RD, tag="pt")
                            # exp(s/8) fused into the PSUM->SBUF copy
                            nc.scalar.activation(pt[:], sg[:], Exp, scale=0.125)
                            if grp >= ngrp - 2:
                                j0 = 2 * (grp - (ngrp - 2))
                                nc.vector.tensor_mul(
                                    pt[:], pt[:], mask_sb[:, j0 : j0 + 2, :]
                                )
                            for j in range(2):
                                kb = 2 * grp + j
                                nc.tensor.matmul(
                                    ot[0:65, :],
                                    (vall[:, kb, h * 65 : (h + 1) * 65]),
                                    (pt[:, j, :]),
                                    start=(kb == 0),
                                    stop=(kb == nkb - 1),
                                )
                        # normalize: z_T = O_T[0:64] * (1 / O_T[64]) bcast
                        rinv = nrm.tile([65, QC], RD, tag="rinv")
                        nc.vector.reciprocal(rinv[64:65, :], ot[64:65, :])
                        bc = spool.tile([P, 2, QC], F32, tag="sg")
                        nc.tensor.matmul(
                            bc[0:64, 0, :],
                            (ones_sb[64:65, 0:HD]),
                            (rinv[64:65, :]),
                            start=True,
                            stop=True,
                        )
                        bcs = nrm.tile([HD, QC], RD, tag="bcs")
                        nc.vector.tensor_copy(bcs[:], bc[0:64, 0, :])
                        nc.vector.tensor_mul(
                            zt[h][:, c * QC : (c + 1) * QC], ot[0:64, :], bcs[:]
                        )

        # ---- phase 3: output projection ---------------------------------
        with (
            tc.tile_pool(name="wpsum", bufs=4, space="PSUM") as wpsum,
            tc.tile_pool(name="osb", bufs=4) as osb,
        ):
            for qb in range(SB):
                for nch in range(2):
                    ps = wpsum.tile([P, QC], F32, tag="wp")
                    for h in range(NH):
                        nc.tensor.matmul(
                            ps[:],
                            (zt[h][:, qb * P : (qb + 1) * P]),
                            (wo_sb[:, h, nch * QC : (nch + 1) * QC]),
                            start=(h == 0),
                            stop=(h == NH - 1),
                        )
                    ob = osb.tile([P, QC], F32, tag="ob")
                    nc.vector.tensor_copy(ob[:], ps[:])
                    nc.sync.dma_start(
                        out_d[qb * P : (qb + 1) * P, nch * QC : (nch + 1) * QC],
                        ob[:],
                    )


def make_mask():
    """mask[p, j*512 + q] = 1.0 iff (j*128 + p) <= q  (causal, diag band)."""
    p = np.arange(P)[:, None, None]
    j = np.arange(4)[None, :, None]
    q = np.arange(QC)[None, None, :]
    return ((j * P + p) <= q).astype(np.float32).reshape(P, 4 * QC)


_NC = None


def _get_nc():
    global _NC
    if _NC is None:
        _NC = build_nc()
    return _NC


def make_in_maps(x, Wq, Wk, Wv, Wo):
    mask = make_mask()
    in_maps = []
    for core in range(8):
        b, hg = divmod(core, 4)
        cols = slice(hg * NH * HD, (hg + 1) * NH * HD)
        in_maps.append(
            {
                "xT": np.ascontiguousarray(np.asarray(x[b], np.float32).T),
                "wq": np.ascontiguousarray(np.asarray(Wq, np.float32)[:, cols]),
                "wk": np.ascontiguousarray(np.asarray(Wk, np.float32)[:, cols]),
                "wv": np.ascontiguousarray(np.asarray(Wv, np.float32)[:, cols]),
                "wo": np.ascontiguousarray(np.asarray(Wo, np.float32)[cols, :]),
                "mask": mask,
            }
        )
    return in_maps


def kernel(x, Wq, Wk, Wv, Wo, bo):
    from concourse.bass_utils import run_bass_kernel_spmd

    nc = _get_nc()
    in_maps = make_in_maps(x, Wq, Wk, Wv, Wo)
    res = run_bass_kernel_spmd(nc, in_maps, list(range(8))).results
    parts = [r["out"] for r in res]
    out = np.stack(
        [
            parts[0] + parts[1] + parts[2] + parts[3],
            parts[4] + parts[5] + parts[6] + parts[7],
        ]
    )
    return (out + np.asarray(bo, np.float32)).astype(np.float32)


# revision 8
# speedup vs baseline: 1.2501x; 1.2501x over previous
"""Multi-head causal attention (B=2, S=2048, D=1024, H=16) on 8 TRN2 cores.

Sharding: data-parallel over batch (2) x tensor-parallel over heads (4 groups
of 4 heads). Core i handles batch i//4, heads 4*(i%4) .. 4*(i%4)+4. Each core
computes the partial output sum_{its heads} softmax(mask(q k^T)/8) v @ Wo_rows
as a full [2048, 1024] array; the host sums the 4 partials per batch and adds
the bias.

Device kernel (identical SPMD program, per-core data):
  inputs : xT [1024, 2048] (x[b] transposed), wq/wk/wv [1024, 256] (head cols),
           wo [256, 1024] (head rows), mask [128, 2048] (causal 0/1 diag band)
  output : out [2048, 1024] partial

All matmuls run as float32r (full-rate fp32 PE mode). Attention works in the
transposed layout S_T[kj, qi] so that:
  - scores matmul: lhsT = K_T block [64, 128], rhs = Q_T chunk [64, 512]
  - softmax exp is fused into the PSUM->SBUF copy on the scalar engine
    (exp(s/8), no max subtraction needed: |s|/8 <= ~3 for this data scale)
  - P @ V needs no transposes: out O_T[d, qi] = sum_kb V[kb]^T P_T[kb]
  - the softmax denominator comes free as an extra ones-column in V
  - normalization is a per-qi-column scale, broadcast across partitions with a
    K=1 outer-product matmul, applied on the PSUM->SBUF copy of O_T
"""

import os
import sys

for _p in ("/opt/trn_rl_repo", "/root/.axon_site/_ro/trn_rl_repo"):
    if os.path.isdir(_p) and _p not in sys.path:
        sys.path.insert(0, _p)

import numpy as np

import concourse.bass as bass
import concourse.tile as tile
from concourse import bacc
from concourse import mybir
from concourse.mybir import dt

# Problem shape (hardcoded per contract)
B, S, D, H = 2, 2048, 1024, 16
HD = D // H          # 64 head dim
NH = 4               # heads per core
P = 128              # partitions
KSUB = D // P        # 8 contraction subtiles
SB = S // P          # 16 seq blocks
QC = 512             # qi chunk width
NCH = S // QC        # 4 qi chunks
VW = 65 * NH         # v_all row width: per head [V(64) | ones(1)]

RD = dt.float32r   # "rounded" fp32: full-rate PE mode; dt.float32 = exact but 4x slower

F32 = dt.float32
Exp = mybir.ActivationFunctionType.Exp


def build_nc():
    nc = bacc.Bacc("TRN2", target_bir_lowering=False, debug=False)

    xT_d = nc.declare_dram_parameter("xT", [D, S], RD, isOutput=False)
    wq_d = nc.declare_dram_parameter("wq", [D, NH * HD], RD, isOutput=False)
    wk_d = nc.declare_dram_parameter("wk", [D, NH * HD], RD, isOutput=False)
    wv_d = nc.declare_dram_parameter("wv", [D, NH * HD], RD, isOutput=False)
    wo_d = nc.declare_dram_parameter("wo", [NH * HD, D], RD, isOutput=False)
    mask_d = nc.declare_dram_parameter("mask", [P, 4 * QC], F32, isOutput=False)
    out_d = nc.declare_dram_parameter("out", [S, D], F32, isOutput=True)

    with tile.TileContext(nc) as tc:
        with nc.allow_low_precision(reason="float32r matmul fast path"):
            _build_body(tc, xT_d, wq_d, wk_d, wv_d, wo_d, mask_d, out_d)
    nc.compile()
    return nc


def _build_body(tc, xT_d, wq_d, wk_d, wv_d, wo_d, mask_d, out_d):
    nc = tc.nc
    from contextlib import ExitStack

    with ExitStack() as ctx:
        persist = ctx.enter_context(tc.tile_pool(name="persist", bufs=1))
        work = ctx.enter_context(tc.tile_pool(name="work", bufs=1))

        # ---- persistent tiles -------------------------------------------
        # Q_T / K_T per head pair g: [128, S]; partitions [0:64] = head 2g,
        # [64:128] = head 2g+1 (falls straight out of the projection matmul).
        qt = [persist.tile([P, S], RD, tag=f"qt{g}", name=f"qt{g}") for g in range(2)]
        kt = [persist.tile([P, S], RD, tag=f"kt{g}", name=f"kt{g}") for g in range(2)]
        # V layout [kj part, sblock, head-interleaved cols]: per head
        # [V_h (64) | ones (1)] so the PV matmul's lhsT [128, 65] yields
        # O_T rows 0:64 = z_h^T and row 64 = softmax denominator.
        vall = persist.tile([P, SB, VW], RD, tag="vall")
        # z^T per head pair [128, S]: lanes 0:64 = head 2g, 64:128 = head 2g+1
        zti = [persist.tile([P, S], RD, tag=f"zti{g}", name=f"zti{g}") for g in range(2)]
        mask_sb = persist.tile([P, 4, QC], F32, tag="mask")
        wo_sb = persist.tile([P, 2, D], RD, tag="wo")
        ones_sb = persist.tile([P, HD], RD, tag="ones")

        # memset cannot emit float32r; stage 1.0 in f32 and round via DVE copy
        onesf = persist.tile([P, 1], F32, tag="onesf")
        nc.vector.memset(onesf[:], 1.0)
        nc.vector.tensor_copy(ones_sb[:], onesf[:].to_broadcast((P, HD)))
        nc.vector.tensor_copy(
            vall[:].rearrange("p s (h c) -> p s h c", c=65)[:, :, :, 64],
            onesf[:].to_broadcast((P, SB, NH)),
        )
        nc.sync.dma_start(mask_sb[:], mask_d[:].rearrange("p (j q) -> p j q", q=QC))
        nc.sync.dma_start(wo_sb[:], wo_d[:].rearrange("(g p) n -> p g n", p=P))

        # ---- phase 1: projections ---------------------------------------
        with (
            tc.tile_pool(name="pin", bufs=1) as pin,
            tc.tile_pool(name="ppsum", bufs=4, space="PSUM") as ppsum,
        ):
            xt_sb = pin.tile([P, KSUB, S], RD, tag="xt")
            for ko in range(KSUB):
                nc.sync.dma_start(
                    xt_sb[:, ko, :], xT_d[ko * P : (ko + 1) * P, :]
                )
            w_sb = {}
            for name, d_ap in (("wq", wq_d), ("wk", wk_d), ("wv", wv_d)):
                w = pin.tile([P, KSUB, NH * HD], RD, tag=name)
                nc.sync.dma_start(
                    w[:], d_ap[:].rearrange("(ko p) c -> p ko c", p=P)
                )
                w_sb[name] = w

            # Q_T, K_T: out [128 (2 heads), s chunk 512] = W_pair^T @ xT
            for dst, wname in ((qt, "wq"), (kt, "wk")):
                w = w_sb[wname]
                for g in range(2):
                    for sc in range(NCH):
                        ps = ppsum.tile([P, QC], F32, tag="pp")
                        for ko in range(KSUB):
                            nc.tensor.matmul(
                                ps[:],
                                (w[:, ko, g * P : (g + 1) * P]),
                                (xt_sb[:, ko, sc * QC : (sc + 1) * QC]),
                                start=(ko == 0),
                                stop=(ko == KSUB - 1),
                            )
                        nc.vector.tensor_copy(
                            dst[g][:, sc * QC : (sc + 1) * QC], ps[:]
                        )

            # V: out [s block 128, 4*64] = xT_block^T @ Wv, scattered into
            # the 65-stride vall layout.
            wv = w_sb["wv"]
            for sb in range(SB):
                ps = ppsum.tile([P, NH * HD], F32, tag="pv")
                for ko in range(KSUB):
                    nc.tensor.matmul(
                        ps[:],
                        (xt_sb[:, ko, sb * P : (sb + 1) * P]),
                        (wv[:, ko, :]),
                        start=(ko == 0),
                        stop=(ko == KSUB - 1),
                    )
                nc.vector.tensor_copy(
                    vall[:, sb, :].rearrange("p (h c) -> p h c", c=65)[:, :, 0:64],
                    ps[:].rearrange("p (h c) -> p h c", c=HD),
                )

        # ---- phase 2: attention -----------------------------------------
        with (
            tc.tile_pool(name="spool", bufs=2, space="PSUM") as spool,
            tc.tile_pool(name="opool", bufs=2, space="PSUM") as opool,
            tc.tile_pool(name="ptpool", bufs=4) as ptpool,
            tc.tile_pool(name="nrm", bufs=2) as nrm,
        ):
            for g in range(2):
                for c in range(NCH):
                    nkb = 4 * (c + 1)          # causal: kj blocks 0..nkb-1
                    qs = slice(c * QC, (c + 1) * QC)
                    ote = opool.tile([P, QC], F32, tag="ote")
                    oto = opool.tile([P, QC], F32, tag="oto")
                    for kb in range(nkb):
                        ks = slice(kb * P, (kb + 1) * P)
                        sg = spool.tile([P, 2, QC], F32, tag="sg")
                        # the two heads' score matmuls run on disjoint PE row
                        # groups (partitions 0:64 / 64:128) -> concurrent
                        for hl in range(2):
                            hs = slice(hl * HD, (hl + 1) * HD)
                            nc.tensor.matmul(
                                sg[:, hl, :],
                                kt[g][hs, ks],
                                qt[g][hs, qs],
                                start=True,
                                stop=True,
                            )
                        pt = ptpool.tile([P, 2, QC], RD, tag="pt")
                        nc.scalar.activation(pt[:], sg[:], Exp, scale=0.125)
                        if kb >= nkb - 4:
                            j = kb - (nkb - 4)
                            nc.vector.tensor_mul(
                                pt[:],
                                pt[:],
                                mask_sb[:, j, None, :].to_broadcast((P, 2, QC)),
                            )
                        for hl, ot in ((0, ote), (1, oto)):
                            h = 2 * g + hl
                            nc.tensor.matmul(
                                ot[0:65, :],
                                vall[:, kb, h * 65 : (h + 1) * 65],
                                pt[:, hl, :],
                                start=(kb == 0),
                                stop=(kb == nkb - 1),
                                skip_group_check=True,
                            )
                    # normalize: z_T = O_T[0:64] * (1 / O_T[64]) bcast; the odd
                    # head lands in zti lanes 64:128 via a small sbuf->sbuf DMA
                    rinv = nrm.tile([65, 2, QC], RD, tag="rinv")
                    nc.vector.reciprocal(rinv[64:65, 0, :], ote[64:65, :])
                    nc.vector.reciprocal(rinv[64:65, 1, :], oto[64:65, :])
                    bc = spool.tile([P, 2, QC], F32, tag="sg")
                    for hl in range(2):
                        nc.tensor.matmul(
                            bc[0:64, hl, :],
                            ones_sb[64:65, 0:HD],
                            rinv[64:65, hl, :],
                            start=True,
                            stop=True,
                        )
                    bcs = nrm.tile([HD, 2, QC], RD, tag="bcs")
                    nc.vector.tensor_copy(bcs[:], bc[0:64, :, :])
                    nc.vector.tensor_mul(zti[g][0:64, qs], ote[0:64, :], bcs[:, 0, :])
                    zos = nrm.tile([HD, QC], RD, tag="zos")
                    nc.vector.tensor_mul(zos[:], oto[0:64, :], bcs[:, 1, :])
                    nc.sync.dma_start(zti[g][64:128, qs], zos[:])

        # ---- phase 3: output projection ---------------------------------
        with (
            tc.tile_pool(name="wpsum", bufs=4, space="PSUM") as wpsum,
            tc.tile_pool(name="osb", bufs=4) as osb,
        ):
            for qb in range(SB):
                for nch in range(2):
                    ps = wpsum.tile([P, QC], F32, tag="wp")
                    for g in range(2):
                        nc.tensor.matmul(
                            ps[:],
                            zti[g][:, qb * P : (qb + 1) * P],
                            wo_sb[:, g, nch * QC : (nch + 1) * QC],
                            start=(g == 0),
                            stop=(g == 1),
                        )
                    ob = osb.tile([P, QC], F32, tag="ob")
                    nc.vector.tensor_copy(ob[:], ps[:])
                    nc.sync.dma_start(
                        out_d[qb * P : (qb + 1) * P, nch * QC : (nch + 1) * QC],
                        ob[:],
                    )


def make_mask():
    """mask[p, j*512 + q] = 1.0 iff (j*128 + p) <= q  (causal, diag band)."""
    p = np.arange(P)[:, None, None]
    j = np.arange(4)[None, :, None]
    q = np.arange(QC)[None, None, :]
    return ((j * P + p) <= q).astype(np.float32).reshape(P, 4 * QC)


_NC = None


def _get_nc():
    global _NC
    if _NC is None:
        _NC = build_nc()
    return _NC


def make_in_maps(x, Wq, Wk, Wv, Wo):
    mask = make_mask()
    in_maps = []
    for core in range(8):
        b, hg = divmod(core, 4)
        cols = slice(hg * NH * HD, (hg + 1) * NH * HD)
        in_maps.append(
            {
                "xT": np.ascontiguousarray(np.asarray(x[b], np.float32).T),
                "wq": np.ascontiguousarray(np.asarray(Wq, np.float32)[:, cols]),
                "wk": np.ascontiguousarray(np.asarray(Wk, np.float32)[:, cols]),
                "wv": np.ascontiguousarray(np.asarray(Wv, np.float32)[:, cols]),
                "wo": np.ascontiguousarray(np.asarray(Wo, np.float32)[cols, :]),
                "mask": mask,
            }
        )
    return in_maps


def kernel(x, Wq, Wk, Wv, Wo, bo):
    from concourse.bass_utils import run_bass_kernel_spmd

    nc = _get_nc()
    in_maps = make_in_maps(x, Wq, Wk, Wv, Wo)
    res = run_bass_kernel_spmd(nc, in_maps, list(range(8))).results
    parts = [r["out"] for r in res]
    out = np.stack(
        [
            parts[0] + parts[1] + parts[2] + parts[3],
            parts[4] + parts[5] + parts[6] + parts[7],
        ]
    )
    return (out + np.asarray(bo, np.float32)).astype(np.float32)


# revision 10
# speedup vs baseline: 1.5096x; 1.2076x over previous
"""Multi-head causal attention (B=2, S=2048, D=1024, H=16) on 8 TRN2 cores.

Sharding: data-parallel over batch (2) x tensor-parallel over heads (4 groups
of 4 heads). Core i handles batch i//4, heads 4*(i%4) .. 4*(i%4)+4. Each core
computes the partial output sum_{its heads} softmax(mask(q k^T)/8) v @ Wo_rows
as a full [2048, 1024] array; the host sums the 4 partials per batch and adds
the bias.

Device kernel (identical SPMD program, per-core data):
  inputs : xT [1024, 2048] (x[b] transposed), wq/wk/wv [1024, 256] (head cols),
           wo [256, 1024] (head rows), mask [128, 2048] (causal 0/1 diag band)
  output : out [2048, 1024] partial

All matmuls run as float32r (full-rate fp32 PE mode). Attention works in the
transposed layout S_T[kj, qi] so that:
  - scores matmul: lhsT = K_T block [64, 128], rhs = Q_T chunk [64, 512]
  - softmax exp is fused into the PSUM->SBUF copy on the scalar engine
    (exp(s/8), no max subtraction needed: |s|/8 <= ~3 for this data scale)
  - P @ V needs no transposes: out O_T[d, qi] = sum_kb V[kb]^T P_T[kb]
  - the softmax denominator comes free as an extra ones-column in V
  - normalization is a per-qi-column scale, broadcast across partitions with a
    K=1 outer-product matmul, applied on the PSUM->SBUF copy of O_T
"""

import os
import sys

for _p in ("/opt/trn_rl_repo", "/root/.axon_site/_ro/trn_rl_repo"):
    if os.path.isdir(_p) and _p not in sys.path:
        sys.path.insert(0, _p)

import numpy as np

import concourse.bass as bass
import concourse.tile as tile
from concourse import bacc
from concourse import mybir
from concourse.mybir import dt

# Problem shape (hardcoded per contract)
B, S, D, H = 2, 2048, 1024, 16
HD = D // H          # 64 head dim
NH = 4               # heads per core
P = 128              # partitions
KSUB = D // P        # 8 contraction subtiles
SB = S // P          # 16 seq blocks
QC = 512             # qi chunk width
NCH = S // QC        # 4 qi chunks
VW = 65 * NH         # v_all row width: per head [V(64) | ones(1)]

RD = dt.float32r   # "rounded" fp32: full-rate PE mode; dt.float32 = exact but 4x slower

F32 = dt.float32
Exp = mybir.ActivationFunctionType.Exp
Ln = mybir.ActivationFunctionType.Ln


def build_nc():
    nc = bacc.Bacc("TRN2", target_bir_lowering=False, debug=False)

    xT_d = nc.declare_dram_parameter("xT", [D, S], RD, isOutput=False)
    wq_d = nc.declare_dram_parameter("wq", [D, NH * HD], RD, isOutput=False)
    wk_d = nc.declare_dram_parameter("wk", [D, NH * HD], RD, isOutput=False)
    wv_d = nc.declare_dram_parameter("wv", [D, NH * HD], RD, isOutput=False)
    wo_d = nc.declare_dram_parameter("wo", [NH * HD, D], RD, isOutput=False)
    mask_d = nc.declare_dram_parameter("mask", [P, 4 * QC], F32, isOutput=False)
    out_d = nc.declare_dram_parameter("out", [S, D], F32, isOutput=True)

    with tile.TileContext(nc) as tc:
        with nc.allow_low_precision(reason="float32r matmul fast path"):
            _build_body(tc, xT_d, wq_d, wk_d, wv_d, wo_d, mask_d, out_d)
    nc.compile()
    return nc


def _build_body(tc, xT_d, wq_d, wk_d, wv_d, wo_d, mask_d, out_d):
    nc = tc.nc
    from contextlib import ExitStack

    with ExitStack() as ctx:
        persist = ctx.enter_context(tc.tile_pool(name="persist", bufs=1))
        work = ctx.enter_context(tc.tile_pool(name="work", bufs=1))

        # ---- persistent tiles -------------------------------------------
        # Q_T / K_T per head pair g: [128, S]; partitions [0:64] = head 2g,
        # [64:128] = head 2g+1 (falls straight out of the projection matmul).
        qt = [persist.tile([P, S], RD, tag=f"qt{g}", name=f"qt{g}") for g in range(2)]
        kt = [persist.tile([P, S], RD, tag=f"kt{g}", name=f"kt{g}") for g in range(2)]
        # V layout [kj part, sblock, head-interleaved cols]: per head
        # [V_h (64) | ones (1)] so the PV matmul's lhsT [128, 65] yields
        # O_T rows 0:64 = z_h^T and row 64 = softmax denominator.
        vall = persist.tile([P, SB, VW], RD, tag="vall")
        # z^T per head pair [128, S]: lanes 0:64 = head 2g, 64:128 = head 2g+1
        zti = [persist.tile([P, S], RD, tag=f"zti{g}", name=f"zti{g}") for g in range(2)]
        mask_sb = persist.tile([P, 4, QC], F32, tag="mask")
        wo_sb = persist.tile([P, 2, D], RD, tag="wo")
        ones_sb = persist.tile([P, HD], RD, tag="ones")

        # memset cannot emit float32r; stage 1.0 in f32 and round via DVE copy
        onesf = persist.tile([P, 1], F32, tag="onesf")
        nc.vector.memset(onesf[:], 1.0)
        nc.vector.tensor_copy(ones_sb[:], onesf[:].to_broadcast((P, HD)))
        nc.vector.tensor_copy(
            vall[:].rearrange("p s (h c) -> p s h c", c=65)[:, :, :, 64],
            onesf[:].to_broadcast((P, SB, NH)),
        )

        # ---- phase 1: projections ---------------------------------------
        with (
            tc.tile_pool(name="pin", bufs=1) as pin,
            tc.tile_pool(name="ppsum", bufs=4, space="PSUM") as ppsum,
        ):
            w_sb = {}
            for name, d_ap in (("wq", wq_d), ("wk", wk_d), ("wv", wv_d)):
                w = pin.tile([P, KSUB, NH * HD], RD, tag=name)
                nc.sync.dma_start(
                    w[:], d_ap[:].rearrange("(ko p) c -> p ko c", p=P)
                )
                w_sb[name] = w
            # one tile per ko block: matmul ko only waits on its own DMA
            xt_sb = [
                pin.tile([P, S], RD, tag=f"xt{ko}", name=f"xt{ko}")
                for ko in range(KSUB)
            ]
            for ko in range(KSUB):
                nc.sync.dma_start(xt_sb[ko][:], xT_d[ko * P : (ko + 1) * P, :])
            nc.sync.dma_start(
                mask_sb[:], mask_d[:].rearrange("p (j q) -> p j q", q=QC)
            )
            nc.sync.dma_start(wo_sb[:], wo_d[:].rearrange("(g p) n -> p g n", p=P))

            # Q_T, K_T: out [128 (2 heads), s chunk 512] = W_pair^T @ xT
            for dst, wname in ((qt, "wq"), (kt, "wk")):
                w = w_sb[wname]
                for g in range(2):
                    for sc in range(NCH):
                        ps = ppsum.tile([P, QC], F32, tag="pp")
                        for ko in range(KSUB):
                            nc.tensor.matmul(
                                ps[:],
                                (w[:, ko, g * P : (g + 1) * P]),
                                xt_sb[ko][:, sc * QC : (sc + 1) * QC],
                                start=(ko == 0),
                                stop=(ko == KSUB - 1),
                            )
                        nc.vector.tensor_copy(
                            dst[g][:, sc * QC : (sc + 1) * QC], ps[:]
                        )

            # V: out [s block 128, 4*64] = xT_block^T @ Wv, scattered into
            # the 65-stride vall layout.
            wv = w_sb["wv"]
            for sb in range(SB):
                ps = ppsum.tile([P, NH * HD], F32, tag="pv")
                for ko in range(KSUB):
                    nc.tensor.matmul(
                        ps[:],
                        xt_sb[ko][:, sb * P : (sb + 1) * P],
                        (wv[:, ko, :]),
                        start=(ko == 0),
                        stop=(ko == KSUB - 1),
                    )
                nc.vector.tensor_copy(
                    vall[:, sb, :].rearrange("p (h c) -> p h c", c=65)[:, :, 0:64],
                    ps[:].rearrange("p (h c) -> p h c", c=HD),
                )

        # ---- phase 2: attention -----------------------------------------
        with (
            tc.tile_pool(name="spool", bufs=2, space="PSUM") as spool,
            tc.tile_pool(name="opool", bufs=2, space="PSUM") as opool,
            tc.tile_pool(name="ptpool", bufs=4) as ptpool,
            tc.tile_pool(name="nrm", bufs=2) as nrm,
        ):
            for g in range(2):
                for c in range(NCH):
                    nkb = 4 * (c + 1)          # causal: kj blocks 0..nkb-1
                    qs = slice(c * QC, (c + 1) * QC)
                    ote = opool.tile([P, QC], F32, tag="ote")
                    oto = opool.tile([P, QC], F32, tag="oto")
                    for kb in range(nkb):
                        ks = slice(kb * P, (kb + 1) * P)
                        sg = spool.tile([P, 2, QC], F32, tag="sg")
                        # the two heads' score matmuls run on disjoint PE row
                        # groups (partitions 0:64 / 64:128) -> concurrent
                        for hl in range(2):
                            hs = slice(hl * HD, (hl + 1) * HD)
                            nc.tensor.matmul(
                                sg[:, hl, :],
                                kt[g][hs, ks],
                                qt[g][hs, qs],
                                start=True,
                                stop=True,
                            )
                        pt = ptpool.tile([P, 2, QC], RD, tag="pt")
                        nc.scalar.activation(pt[:], sg[:], Exp, scale=0.125)
                        if kb >= nkb - 4:
                            j = kb - (nkb - 4)
                            nc.vector.tensor_mul(
                                pt[:],
                                pt[:],
                                mask_sb[:, j, None, :].to_broadcast((P, 2, QC)),
                            )
                        for hl, ot in ((0, ote), (1, oto)):
                            h = 2 * g + hl
                            nc.tensor.matmul(
                                ot[0:65, :],
                                vall[:, kb, h * 65 : (h + 1) * 65],
                                pt[:, hl, :],
                                start=(kb == 0),
                                stop=(kb == nkb - 1),
                                skip_group_check=True,
                            )
                    # normalize: z_T = O_T[0:64] * (1 / O_T[64]) bcast.
                    # 1/r = exp(-ln r) on the scalar engine (ln+exp share one
                    # ACT table set; DVE reciprocal costs 3.3us per row). The
                    # odd head lands in zti lanes 64:128 via sbuf->sbuf DMA.
                    lnr = nrm.tile([65, 2, QC], RD, tag="lnr")
                    rinv = nrm.tile([65, 2, QC], RD, tag="rinv")
                    for hl, ot in ((0, ote), (1, oto)):
                        nc.scalar.activation(lnr[64:65, hl, :], ot[64:65, :], Ln)
                        nc.scalar.activation(
                            rinv[64:65, hl, :], lnr[64:65, hl, :], Exp, scale=-1.0
                        )
                    bce = opool.tile([P, QC], F32, tag="ote")
                    bco = opool.tile([P, QC], F32, tag="oto")
                    for hl, bc in ((0, bce), (1, bco)):
                        nc.tensor.matmul(
                            bc[0:HD, :],
                            ones_sb[64:65, 0:HD],
                            rinv[64:65, hl, :],
                            start=True,
                            stop=True,
                        )
                    bcs = nrm.tile([HD, 2, QC], RD, tag="bcs")
                    nc.vector.tensor_copy(bcs[:, 0, :], bce[0:HD, :])
                    nc.vector.tensor_copy(bcs[:, 1, :], bco[0:HD, :])
                    nc.vector.tensor_mul(zti[g][0:64, qs], ote[0:64, :], bcs[:, 0, :])
                    zos = nrm.tile([HD, QC], RD, tag="zos")
                    nc.vector.tensor_mul(zos[:], oto[0:64, :], bcs[:, 1, :])
                    nc.sync.dma_start(zti[g][64:128, qs], zos[:])

        # ---- phase 3: output projection ---------------------------------
        with (
            tc.tile_pool(name="wpsum", bufs=4, space="PSUM") as wpsum,
            tc.tile_pool(name="osb", bufs=4) as osb,
        ):
            for qb in range(SB):
                for nch in range(2):
                    ps = wpsum.tile([P, QC], F32, tag="wp")
                    for g in range(2):
                        nc.tensor.matmul(
                            ps[:],
                            zti[g][:, qb * P : (qb + 1) * P],
                            wo_sb[:, g, nch * QC : (nch + 1) * QC],
                            start=(g == 0),
                            stop=(g == 1),
                        )
                    ob = osb.tile([P, QC], F32, tag="ob")
                    nc.vector.tensor_copy(ob[:], ps[:])
                    nc.sync.dma_start(
                        out_d[qb * P : (qb + 1) * P, nch * QC : (nch + 1) * QC],
                        ob[:],
                    )


def make_mask():
    """mask[p, j*512 + q] = 1.0 iff (j*128 + p) <= q  (causal, diag band)."""
    p = np.arange(P)[:, None, None]
    j = np.arange(4)[None, :, None]
    q = np.arange(QC)[None, None, :]
    return ((j * P + p) <= q).astype(np.float32).reshape(P, 4 * QC)


_NC = None


def _get_nc():
    global _NC
    if _NC is None:
        _NC = build_nc()
    return _NC


def make_in_maps(x, Wq, Wk, Wv, Wo):
    mask = make_mask()
    in_maps = []
    for core in range(8):
        b, hg = divmod(core, 4)
        cols = slice(hg * NH * HD, (hg + 1) * NH * HD)
        in_maps.append(
            {
                "xT": np.ascontiguousarray(np.asarray(x[b], np.float32).T),
                "wq": np.ascontiguousarray(np.asarray(Wq, np.float32)[:, cols]),
                "wk": np.ascontiguousarray(np.asarray(Wk, np.float32)[:, cols]),
                "wv": np.ascontiguousarray(np.asarray(Wv, np.float32)[:, cols]),
                "wo": np.ascontiguousarray(np.asarray(Wo, np.float32)[cols, :]),
                "mask": mask,
            }
        )
    return in_maps


def kernel(x, Wq, Wk, Wv, Wo, bo):
    from concourse.bass_utils import run_bass_kernel_spmd

    nc = _get_nc()
    in_maps = make_in_maps(x, Wq, Wk, Wv, Wo)
    res = run_bass_kernel_spmd(nc, in_maps, list(range(8))).results
    parts = [r["out"] for r in res]
    out = np.stack(
        [
            parts[0] + parts[1] + parts[2] + parts[3],
            parts[4] + parts[5] + parts[6] + parts[7],
        ]
    )
    return (out + np.asarray(bo, np.float32)).astype(np.float32)
